# revision 29
# baseline (speedup 1.0000x reference)
"""Trainium2 Bass kernel for nn_ClawMatrix (cross-modal claw-matrix alignment).

reference computation per batch element b:
    vp = relu(LN(v @ vW + vb) * vg + vbeta)          [S, D]
    lp = relu(LN(l @ lW + lb) * lg + lbeta)          [S, D]
    sim = vp @ lp.T * (mean(claw) / 0.07)            [S, S]
    A   = softmax(sim, axis=-1)
    av  = A @ vp                                     [S, D]
    al  = A.T @ lp                                   [S, D]
    out = relu(LN([av, al] @ oW + ob) * og + obeta)  [S, D]

Sharding: data-parallel over batch B=8 across the 8 NeuronCores (one batch
element per core, weights replicated, no collectives).

Device-side strategy (per core):
  - projections / sim / A@vp / output projection in bf16 (fp32 PSUM); the
    A^T@lp bmm runs in fp8e4m3 with perf_mode=DoubleRow (the exp writes E in
    fp8 with a per-256-block t-pair-interleaved layout so the DoubleRow
    ifmap streams it as plain 3D blocks; lp is folded with 1024/Z against
    fp8 underflow and the 1024 divided back out on PSUM evacuation)
  - the LN row mean comes for free out of the matmul: the host appends a
    precomputed row-mean column to each weight matrix, so PSUM column D
    holds mean(y) and the on-chip LN needs only E[y^2] (one fused Square
    activation with accumulate) -> var = E[y^2] - mean^2
  - LN tiles processed in pairs: [P,1] scalar chains batched across the
    pair, one relu on VectorE and one fused into a ScalarE activation
  - exp fused with row-sum via activation accum_out; softmax normalizer 1/Z
    folded into lp (for A^T lp) and into row-normalized bf16 E copies that
    feed the E^T XBAR transposes (for A vp) -- no DMAs in the C/D loop, so
    nothing serializes against the transposes' xbar mode
  - phases C (sim+exp) and D (av^T) interleaved, with D lagging one pair of
    row tiles so the E^T transposes pipeline off the PE critical path;
    av^T/al^T are written in-place over the dead columns of vpT/lpT, which
    is exactly the [2D, S] combined^T layout the output projection consumes
  - one unified PSUM pool (slot-level recycling) across all phases -- fresh
    per-phase pools would insert alloc-waits on the full previous phase
  - SBUF pressure handled with explicitly managed (non-LIFO) pool lifetimes
"""

import os
import sys
import numpy as np

for _p in ("/opt/trn_rl_repo", "/root/.axon_site/_ro/trn_rl_repo"):
    if os.path.isdir(_p) and _p not in sys.path:
        sys.path.insert(0, _p)

import ml_dtypes  # noqa: E402

BF16 = ml_dtypes.bfloat16

P = 128           # partitions
B = 8             # batch / cores
S = 2048          # sequence
D = 768           # feature dim
DPAD = 8          # extra weight columns: [mean col | zero pad]
EPS = 1e-5
TEMPERATURE = 0.07

_BUILD_CACHE = {}


class _Pool:
    """Manually managed tile-pool lifetime (enter now, exit at any point)."""

    def __init__(self, tc, **kw):
        self._cm = tc.tile_pool(**kw)
        self.pool = self._cm.__enter__()
        self._open = True

    def tile(self, *a, **kw):
        if "name" not in kw:
            kw["name"] = kw.get("tag") or f"t{id(self) % 9973}"
        return self.pool.tile(*a, **kw)

    def close(self):
        if self._open:
            self._cm.__exit__(None, None, None)
            self._open = False


def _build(c_scale: float, trivial: bool, s: int = S, d: int = D):
    """Builds the single-core Bass program. Returns the compiled Bacc module."""
    import concourse.bass as bass
    import concourse.tile as tile
    from concourse import bacc, mybir

    f32 = mybir.dt.float32
    bf16 = mybir.dt.bfloat16
    f8 = mybir.dt.float8e4
    DR = mybir.MatmulPerfMode.DoubleRow
    AF = mybir.ActivationFunctionType
    AX = mybir.AxisListType
    OP = mybir.AluOpType

    st_n = s // P          # number of 128-row tiles over S
    dt_n = d // P          # number of 128-row tiles over D
    kt_n = 2 * dt_n        # k tiles over 2D for the output projection
    ch = 512               # matmul free-dim chunk (one PSUM bank of fp32)
    tc_n = s // ch         # chunks over S
    g_sz = ch // P         # s-tiles per 512-column group
    dw = d + DPAD          # weight width incl. mean column + pad
    d_chunks = [(i, min(ch, dw - i)) for i in range(0, dw, ch)]

    nc = bacc.Bacc(
        "TRN2",
        target_bir_lowering=False,
        debug=False,
        enable_asserts=False,
        num_devices=B,
    )

    vlT_d = nc.dram_tensor("vlT", [2 * d, s], bf16, kind="ExternalInput")
    vW_d = nc.dram_tensor("vW", [d, dw], bf16, kind="ExternalInput")
    lW_d = nc.dram_tensor("lW", [d, dw], bf16, kind="ExternalInput")
    oW_d = nc.dram_tensor("oW", [2 * d, dw], bf16, kind="ExternalInput")
    if not trivial:
        vb_d = nc.dram_tensor("vb", [1, dw], bf16, kind="ExternalInput")
        lb_d = nc.dram_tensor("lb", [1, dw], bf16, kind="ExternalInput")
        ob_d = nc.dram_tensor("ob", [1, dw], bf16, kind="ExternalInput")
        vg_d = nc.dram_tensor("vg", [1, d], f32, kind="ExternalInput")
        vbe_d = nc.dram_tensor("vbeta", [1, d], f32, kind="ExternalInput")
        lg_d = nc.dram_tensor("lg", [1, d], f32, kind="ExternalInput")
        lbe_d = nc.dram_tensor("lbeta", [1, d], f32, kind="ExternalInput")
        og_d = nc.dram_tensor("og", [1, d], f32, kind="ExternalInput")
        obe_d = nc.dram_tensor("obeta", [1, d], f32, kind="ExternalInput")
    out_d = nc.dram_tensor("out", [s, d], f32, kind="ExternalOutput")

    with tile.TileContext(nc) as tc:
        pp = _Pool(tc, name="persist", bufs=1)
        sp = _Pool(tc, name="small", bufs=4)

        eps_sb = pp.tile([P, 1], f32, tag='eps')
        nc.vector.memset(eps_sb[:], EPS)
        racc = pp.tile([P, st_n, tc_n], f32, tag='racc')
        rinv_all = pp.tile([P, st_n], f32, tag='rinv_all')
        sqt_p = _Pool(tc, name="sqt", bufs=3)

        if not trivial:
            ones_sb = pp.tile([1, P], bf16, tag="ones_sb")
            nc.vector.memset(ones_sb[:], 1.0)
            b_sb = {}
            aff = {}
            for nm, dd in (("vb", vb_d), ("lb", lb_d), ("ob", ob_d)):
                t = pp.tile([1, dw], bf16, tag=nm)
                nc.sync.dma_start(out=t[:], in_=dd.ap())
                b_sb[nm] = t
            for nm, dd in (("vg", vg_d), ("vbeta", vbe_d), ("lg", lg_d),
                           ("lbeta", lbe_d), ("og", og_d), ("obeta", obe_d)):
                t = pp.tile([P, d], f32, tag=nm)
                src = bass.AP(tensor=dd.ap().tensor, offset=0,
                              ap=[[0, P], [1, d]])
                nc.sync.dma_start(out=t[:], in_=src)
                aff[nm] = t

        # Address reuse via same-tag slot cycling (bufs=1):
        #   slab: vlT_sb -> E_all
        # pT_all doubles as combined^T: av^T/al^T overwrite the dead columns
        # of vpT/lpT in place (fine-grained WAR tracked by Tile).
        slab = _Pool(tc, name="slab", bufs=1)
        slab48 = _Pool(tc, name="slab48", bufs=1)
        vplp_p = _Pool(tc, name="vplp", bufs=1)
        w_p = _Pool(tc, name="wproj", bufs=2)
        ps_p = _Pool(tc, name="psuni", bufs=4, space=bass.MemorySpace.PSUM)

        def psum_tile(n):
            # one shared slot size (2 banks) for every phase: slot-level
            # recycling instead of pool-boundary barriers
            t = ps_p.tile([P, dw], f32, tag="ps")
            return t[:, :n]

        vp_all = vplp_p.tile([P, st_n, d], bf16, tag='vp_all')
        lp_all = vplp_p.tile([P, st_n, d], bf16, tag='lp_all')
        lp8 = vplp_p.tile([P, st_n, d], f8, tag='lp8')
        pT_all = slab48.tile([P, 2 * dt_n, s], bf16, tag='slab48',
                             name='pT_all')
        vpT_all = pT_all[:, :dt_n, :]
        lpT_all = pT_all[:, dt_n:, :]

        inv_sqrt_d = 1.0 / float(np.sqrt(d))

        def layernorm_relu_pair(pss, dsts, g_nm, be_nm, tagsfx,
                                dst_f32=False):
            """LN+relu over [:, :d] of a pair of psum tiles (mean precomputed
            in column d by the matmul). Small [P,·] scalars are batched
            across the pair to halve fixed op overheads.

            Engine split: ScalarE does the big Square pass (fused E[y^2]
            accumulate) + sqrt; VectorE does the small scalars and the
            scale/bias/relu passes.
            """
            n = len(pss)
            ssq = sp.tile([P, 2], f32, tag="ssq" + tagsfx)
            mcp = sp.tile([P, 2], f32, tag="mcp" + tagsfx)
            sqts = []
            for i, ps in enumerate(pss):
                sqt = sqt_p.tile([P, d], bf16, tag="sqt")
                nc.scalar.activation(out=sqt[:], in_=ps[:, :d],
                                     func=AF.Square, scale=inv_sqrt_d,
                                     accum_out=ssq[:, i:i + 1])
                nc.scalar.activation(out=mcp[:, i:i + 1], in_=ps[:, d:d + 1],
                                     func=AF.Copy)
                sqts.append(sqt)
            var = sp.tile([P, 2], f32, tag="var" + tagsfx)
            nc.vector.tensor_tensor(out=var[:, :n], in0=mcp[:, :n],
                                    in1=mcp[:, :n], op=OP.mult)
            nc.vector.tensor_tensor(out=var[:, :n], in0=ssq[:, :n],
                                    in1=var[:, :n], op=OP.subtract)
            rstd = sp.tile([P, 2], f32, tag="rstd" + tagsfx)
            nc.scalar.activation(out=rstd[:, :n], in_=var[:, :n],
                                 func=AF.Sqrt, bias=eps_sb[:])
            nc.vector.reciprocal(out=rstd[:, :n], in_=rstd[:, :n])
            mr = sp.tile([P, 2], f32, tag="mr" + tagsfx)
            nc.vector.tensor_tensor(out=mr[:, :n], in0=mcp[:, :n],
                                    in1=rstd[:, :n], op=OP.mult)
            nmr = sp.tile([P, 2], f32, tag="nmr" + tagsfx)
            nc.vector.tensor_scalar(out=nmr[:, :n], in0=mr[:, :n],
                                    scalar1=-1.0, scalar2=None, op0=OP.mult)
            for i, (ps, dst) in enumerate(zip(pss, dsts)):
                if trivial:
                    if i % 2 == 1:
                        # odd tile of the pair: fused relu on ScalarE to
                        # halve the VectorE tail latency
                        nc.scalar.activation(out=dst, in_=ps[:, :d],
                                             func=AF.Relu,
                                             bias=nmr[:, i:i + 1],
                                             scale=rstd[:, i:i + 1])
                        continue
                    if dst_f32:
                        tmp = ot_p.tile([P, d], f32, tag="tmpf", bufs=2)
                    else:
                        tmp = sqts[i]
                    nc.vector.tensor_scalar(out=tmp[:], in0=ps[:, :d],
                                            scalar1=rstd[:, i:i + 1],
                                            scalar2=mr[:, i:i + 1],
                                            op0=OP.mult, op1=OP.subtract)
                    nc.vector.tensor_scalar_max(out=dst, in0=tmp[:],
                                                scalar1=0.0)
                else:
                    nrm = sp.tile([P, d], f32, tag="nrm" + tagsfx, bufs=2)
                    nc.vector.tensor_scalar(out=nrm[:], in0=ps[:, :d],
                                            scalar1=rstd[:, i:i + 1],
                                            scalar2=mr[:, i:i + 1],
                                            op0=OP.mult, op1=OP.subtract)
                    nc.vector.tensor_mul(out=nrm[:], in0=nrm[:],
                                         in1=aff[g_nm][:])
                    nc.vector.tensor_add(out=nrm[:], in0=nrm[:],
                                         in1=aff[be_nm][:])
                    nc.vector.tensor_scalar_max(out=dst, in0=nrm[:],
                                                scalar1=0.0)

        def linear_into_psum(ps, x_sb, W_sb, bias_nm, n_k):
            for c0, cl in d_chunks:
                for kt in range(n_k):
                    nc.tensor.matmul(
                        ps[:, c0:c0 + cl],
                        x_sb(kt),
                        W_sb[:, kt, c0:c0 + cl],
                        start=(kt == 0),
                        stop=(kt == n_k - 1 and trivial),
                    )
                if not trivial:
                    nc.tensor.matmul(
                        ps[:, c0:c0 + cl], ones_sb[:1, :],
                        b_sb[bias_nm][:1, c0:c0 + cl],
                        start=False, stop=True)

        # ---------- phase A/B: projections ----------
        # packed v/l input; first halves of the v k-tiles land first so the
        # first matmul can start ASAP
        vlT_sb = slab.tile([P, 2 * dt_n, s], bf16, tag="slab",
                           name="vlT_sb")
        hc = s // 4

        def load_chunk(j, h):
            nc.sync.dma_start(
                out=vlT_sb[:, j, h * hc:(h + 1) * hc],
                in_=vlT_d.ap()[j * P:(j + 1) * P, h * hc:(h + 1) * hc])

        for j in range(dt_n, 2 * dt_n):
            load_chunk(j, 0)
        # one batch per proj-l pair: l h=1..3, then v h=0..3; each batch
        # lands several pairs before its consumer
        load_batches = [[(j, h) for j in range(dt_n, 2 * dt_n)]
                        for h in range(1, 4)]
        load_batches += [[(j, h) for j in range(dt_n)] for h in range(4)]

        def proj(base, W_d, xp_all, xpT_all, bias_nm, g_nm, be_nm,
                 filler=None):
            W_sb = w_p.tile([P, dt_n, dw], bf16, tag="wproj", name="W_sb")
            for j in range(dt_n):
                nc.gpsimd.dma_start(out=W_sb[:, j, :],
                                    in_=W_d.ap()[j * P:(j + 1) * P, :])
            for st0 in range(0, st_n, 2):
                pss, dsts = [], []
                for st in (st0, st0 + 1):
                    ps = psum_tile(dw)
                    linear_into_psum(
                        ps,
                        lambda kt: vlT_sb[:, base + kt, st * P:(st + 1) * P],
                        W_sb, bias_nm, dt_n)
                    pss.append(ps)
                    dsts.append(xp_all[:, st, :])
                layernorm_relu_pair(pss, dsts, g_nm, be_nm, "p")
                for st in (st0, st0 + 1):
                    nc.sync.dma_start_transpose(
                        out=xpT_all[:, :, st * P:(st + 1) * P],
                        in_=xp_all[:, st, :])
                if filler is not None:
                    filler(st0 // 2)

        def drip_loads(p):
            if p < len(load_batches):
                for j, h in load_batches[p]:
                    load_chunk(j, h)

        proj(dt_n, lW_d, lp_all, lpT_all, "lb", "lg", "lbeta",
             filler=drip_loads)
        proj(0, vW_d, vp_all, vpT_all, "vb", "vg", "vbeta")
        w_p.close()

        # ---------- phase C+D interleaved (D lags one pair) ----------
        # C: sim row-tile st -> E (exp with accumulated row sums); E rows are
        # then rescaled by the softmax normalizer into En (so phase D needs
        # no rinv broadcast at all).
        # D (per pair of row tiles q): E^T transposes of En -> av^T columns
        # written over vpT's dead columns.
        E8 = slab.tile([P, st_n, s], f8, tag="slab", name="E8")
        g2 = 2 * P                    # s-columns per D group (2 row tiles)
        at_p = _Pool(tc, name="at", bufs=2)
        en_p = _Pool(tc, name="en", bufs=2)
        en_tiles = {}
        rinvK = pp.tile([P, st_n], f32, tag='rinvK')

        def phase_c(st):
            for t0 in range(tc_n):
                ps = psum_tile(ch)
                for dt in range(dt_n):
                    nc.tensor.matmul(
                        ps[:],
                        vpT_all[:, dt, st * P:(st + 1) * P],
                        lpT_all[:, dt, t0 * ch:(t0 + 1) * ch],
                        start=(dt == 0), stop=(dt == dt_n - 1))
                # fp8 E, stored per-256-block t-pair-interleaved so the
                # DoubleRow ifmap of phase E reads it as 3D blocks
                eout = E8[:, st, t0 * ch:(t0 + 1) * ch].rearrange(
                    "p (c q two) -> p c two q", c=2, q=P, two=2)
                nc.scalar.activation(
                    out=eout, in_=ps[:], func=AF.Exp, scale=float(c_scale),
                    accum_out=racc[:, st, t0:t0 + 1])
            rs = sp.tile([P, 1], f32, tag="rs")
            nc.vector.tensor_reduce(out=rs[:], in_=racc[:, st, :],
                                    axis=AX.X, op=OP.add)
            nc.vector.reciprocal(out=rinv_all[:, st:st + 1], in_=rs[:])
            nc.vector.tensor_scalar(out=rinvK[:, st:st + 1],
                                    in0=rinv_all[:, st:st + 1],
                                    scalar1=1024.0, scalar2=None, op0=OP.mult)
            # fold scaled-up 1/Z into fp8 lp rows (plain 1/Z would underflow
            # fp8; the al^T evacuation divides the 1024 back out)
            nc.vector.tensor_scalar_mul(
                out=lp8[:, st, :], in0=lp_all[:, st, :],
                scalar1=rinvK[:, st:st + 1])
            # row-normalized bf16 E copy for the A @ vp path
            q = st // 2
            if st % 2 == 0:
                en_tiles[q] = en_p.tile([P, 2, s], bf16, tag="en")
            esrc = E8[:, st, :].rearrange("p (c q two) -> p c two q",
                                          c=s // (2 * P), q=P, two=2)
            edst = en_tiles[q][:, st % 2, :].rearrange(
                "p (c two q) -> p c two q", c=s // (2 * P), two=2, q=P)
            nc.vector.tensor_scalar_mul(
                out=edst, in0=esrc, scalar1=rinv_all[:, st:st + 1])

        def phase_d(q):
            en = en_tiles.pop(q)
            at = at_p.tile([P, st_n, g2], bf16, tag="atg")
            for i in range(2):
                nc.sync.dma_start_transpose(
                    out=at[:, :, i * P:(i + 1) * P],
                    in_=en[:, i, :])
            for dt in range(dt_n):
                ps = psum_tile(g2)
                for tt in range(st_n):
                    nc.tensor.matmul(
                        ps[:],
                        vp_all[:, tt, dt * P:(dt + 1) * P],
                        at[:, tt, :],
                        start=(tt == 0), stop=(tt == st_n - 1))
                # psum -> av^T in vpT's dead columns (already 1/Z-scaled)
                nc.vector.tensor_copy(
                    out=pT_all[:, dt, q * g2:(q + 1) * g2], in_=ps[:])

        for st in range(st_n):
            phase_c(st)
            if st % 2 == 1 and st >= 3:
                phase_d((st - 3) // 2)
        phase_d(st_n // 2 - 1)
        en_p.close()
        at_p.close()

        # ---------- phase E: al^T = (lp')^T E ; phase F: output projection --
        ow_p = _Pool(tc, name="wout", bufs=1)
        ot_p = _Pool(tc, name="outsb", bufs=2)
        oW_sb = ow_p.tile([P, kt_n, dw], bf16, tag="wout", name="oW_sb")
        for j in range(kt_n):
            nc.gpsimd.dma_start(out=oW_sb[:, j, :],
                                in_=oW_d.ap()[j * P:(j + 1) * P, :])

        for t0 in range(tc_n):
            for dt in range(dt_n):
                ps = psum_tile(ch)
                for q in range(st_n // 2):
                    # 3D rhs free pattern: stream the stored (c, 2*q2+two)
                    # order; psum columns come out t-permuted and are
                    # unpermuted by the evacuation AP below
                    erhs = E8[:, 2 * q:2 * q + 2,
                              t0 * ch:(t0 + 1) * ch].rearrange(
                        "p a (c f) -> p a c f", c=2, f=g2)
                    nc.tensor.matmul(
                        ps[:],
                        lp8[:, 2 * q:2 * q + 2, dt * P:(dt + 1) * P],
                        erhs,
                        start=(q == 0), stop=(q == st_n // 2 - 1),
                        perf_mode=DR)
                # al^T over lpT's dead columns; divide out the 1024 from
                # the scaled-up lp fold. Columns stay pair-interleaved
                # (contiguous store); the output projection's lhsT AP
                # unpermutes them.
                nc.vector.tensor_scalar_mul(
                    out=pT_all[:, dt_n + dt, t0 * ch:(t0 + 1) * ch],
                    in0=ps[:], scalar1=1.0 / 1024.0)
            def comb_lhsT(kt, rt):
                if kt < dt_n:
                    return pT_all[:, kt, rt * P:(rt + 1) * P]
                # al half: columns are stored (c, 2*q + two)-interleaved
                # within each 512-chunk
                tq, r = divmod(rt, g_sz)
                cc, two = divmod(r, 2)
                blk = pT_all[:, kt, tq * ch:(tq + 1) * ch].rearrange(
                    "p (c q two) -> p c two q", c=2, q=P, two=2)
                return blk[:, cc, two, :]

            for rt0 in range(t0 * g_sz, (t0 + 1) * g_sz, 2):
                pss, ots = [], []
                for rt in (rt0, rt0 + 1):
                    ps = psum_tile(dw)
                    linear_into_psum(
                        ps, lambda kt, rt=rt: comb_lhsT(kt, rt),
                        oW_sb, "ob", kt_n)
                    pss.append(ps)
                    ots.append(ot_p.tile([P, d], f32, tag="ot"))
                layernorm_relu_pair(pss, [o[:] for o in ots],
                                    "og", "obeta", "o", dst_f32=True)
                for i, rt in enumerate((rt0, rt0 + 1)):
                    nc.sync.dma_start(
                        out=out_d.ap()[rt * P:(rt + 1) * P, :], in_=ots[i][:])
        ot_p.close()
        ow_p.close()
        ps_p.close()
        vplp_p.close()
        slab48.close()
        slab.close()
        sqt_p.close()
        sp.close()
        pp.close()

    nc.compile()
    return nc


def _get_program(c_scale: float, trivial: bool, s: int = S, d: int = D):
    key = (round(float(c_scale), 12), trivial, s, d)
    if key not in _BUILD_CACHE:
        _BUILD_CACHE[key] = _build(c_scale, trivial, s, d)
    return _BUILD_CACHE[key]


def _with_mean_col(W):
    """[K, N] weights -> [K, N + DPAD] bf16 with col N = row-mean, pad 0."""
    W = np.asarray(W, np.float32)
    k = W.shape[0]
    ext = np.zeros((k, W.shape[1] + DPAD), np.float32)
    ext[:, :W.shape[1]] = W
    ext[:, W.shape[1]] = W.mean(axis=1)
    return np.ascontiguousarray(ext.astype(BF16))


def _prep_in_maps(vision, language, vW, lW, oW, c_scale, trivial, extras):
    n_b = vision.shape[0]
    vWb = _with_mean_col(vW)
    lWb = _with_mean_col(lW)
    oWb = _with_mean_col(oW)
    in_maps = []
    for b in range(n_b):
        vlT = np.concatenate([vision[b].T, language[b].T], 0)
        m = {
            "vlT": np.ascontiguousarray(vlT.astype(BF16)),
            "vW": vWb, "lW": lWb, "oW": oWb,
        }
        if not trivial:
            m.update(extras)
        in_maps.append(m)
    return in_maps


def _program_and_inmaps(inputs):
    """(compiled program, per-core input maps) for the given full inputs."""
    vision = np.asarray(inputs["vision_features"], np.float32)
    language = np.asarray(inputs["language_features"], np.float32)
    c_scale = float(np.asarray(inputs["claw"], np.float32).mean()) / TEMPERATURE
    nc = _get_program(c_scale, True)
    in_maps = _prep_in_maps(vision, language, inputs["vW"], inputs["lW"],
                            inputs["oW"], c_scale, True, {})
    return nc, in_maps


def kernel(vision_features, language_features, vW, vb, vg, vbeta,
           lW, lb, lg, lbeta, claw, oW, ob, og, obeta):
    from concourse import bass_utils

    vision = np.asarray(vision_features, np.float32)
    language = np.asarray(language_features, np.float32)
    c_scale = float(np.asarray(claw, np.float32).mean()) / TEMPERATURE
    # softmax is computed without max-subtraction: guard that exp can't
    # overflow (|logit| <= |c| * max|sim|; rows have L2 norm <~ sqrt(D))
    assert abs(c_scale) * 1.5 * D < 80.0, "logit scale too large for exp"

    trivial = (
        np.all(np.asarray(vb) == 0) and np.all(np.asarray(lb) == 0)
        and np.all(np.asarray(ob) == 0)
        and np.all(np.asarray(vg) == 1) and np.all(np.asarray(vbeta) == 0)
        and np.all(np.asarray(lg) == 1) and np.all(np.asarray(lbeta) == 0)
        and np.all(np.asarray(og) == 1) and np.all(np.asarray(obeta) == 0)
    )

    def bias_ext(bv):
        bv = np.asarray(bv, np.float32).reshape(D)
        ext = np.zeros(D + DPAD, np.float32)
        ext[:D] = bv
        ext[D] = bv.mean()
        return ext.reshape(1, D + DPAD).astype(BF16)

    extras = {}
    if not trivial:
        extras = {
            "vb": bias_ext(vb), "lb": bias_ext(lb), "ob": bias_ext(ob),
            "vg": np.asarray(vg, np.float32).reshape(1, D),
            "vbeta": np.asarray(vbeta, np.float32).reshape(1, D),
            "lg": np.asarray(lg, np.float32).reshape(1, D),
            "lbeta": np.asarray(lbeta, np.float32).reshape(1, D),
            "og": np.asarray(og, np.float32).reshape(1, D),
            "obeta": np.asarray(obeta, np.float32).reshape(1, D),
        }

    nc = _get_program(c_scale, trivial)
    in_maps = _prep_in_maps(vision, language, vW, lW, oW,
                            c_scale, trivial, extras)
    res = bass_utils.run_bass_kernel_spmd(nc, in_maps,
                                          core_ids=list(range(B)))
    return np.stack([res.results[b]["out"] for b in range(B)], axis=0)


# revision 30
# speedup vs baseline: 1.1883x; 1.1883x over previous
"""Trainium2 Bass kernel for nn_ClawMatrix (cross-modal claw-matrix alignment).

reference computation per batch element b:
    vp = relu(LN(v @ vW + vb) * vg + vbeta)          [S, D]
    lp = relu(LN(l @ lW + lb) * lg + lbeta)          [S, D]
    sim = vp @ lp.T * (mean(claw) / 0.07)            [S, S]
    A   = softmax(sim, axis=-1)
    av  = A @ vp                                     [S, D]
    al  = A.T @ lp                                   [S, D]
    out = relu(LN([av, al] @ oW + ob) * og + obeta)  [S, D]

Sharding: data-parallel over batch B=8 across the 8 NeuronCores (one batch
element per core, weights replicated, no collectives).

Device-side strategy (per core):
  - projections / sim / A@vp / output projection in bf16 (fp32 PSUM); the
    A^T@lp bmm runs in fp8e4m3 with perf_mode=DoubleRow (the exp writes E in
    fp8 with a per-256-block t-pair-interleaved layout so the DoubleRow
    ifmap streams it as plain 3D blocks; lp is folded with 1024/Z against
    fp8 underflow and the 1024 divided back out on PSUM evacuation)
  - the LN row mean comes for free out of the matmul: the host appends a
    precomputed row-mean column to each weight matrix, so PSUM column D
    holds mean(y) and the on-chip LN needs only E[y^2] (one fused Square
    activation with accumulate) -> var = E[y^2] - mean^2
  - LN tiles processed in pairs: [P,1] scalar chains batched across the
    pair, one relu on VectorE and one fused into a ScalarE activation
  - exp fused with row-sum via activation accum_out; softmax normalizer 1/Z
    folded into lp (for A^T lp) and into row-normalized bf16 E copies that
    feed the E^T XBAR transposes (for A vp) -- no DMAs in the C/D loop, so
    nothing serializes against the transposes' xbar mode
  - phases C (sim+exp) and D (av^T) interleaved, with D lagging one pair of
    row tiles so the E^T transposes pipeline off the PE critical path;
    av^T/al^T are written in-place over the dead columns of vpT/lpT, which
    is exactly the [2D, S] combined^T layout the output projection consumes
  - one unified PSUM pool (slot-level recycling) across all phases -- fresh
    per-phase pools would insert alloc-waits on the full previous phase
  - SBUF pressure handled with explicitly managed (non-LIFO) pool lifetimes
"""

import os
import sys
import numpy as np

for _p in ("/opt/trn_rl_repo", "/root/.axon_site/_ro/trn_rl_repo"):
    if os.path.isdir(_p) and _p not in sys.path:
        sys.path.insert(0, _p)

import ml_dtypes  # noqa: E402

BF16 = ml_dtypes.bfloat16

P = 128           # partitions
B = 8             # batch / cores
S = 2048          # sequence
D = 768           # feature dim
DPAD = 8          # extra weight columns: [mean col | zero pad]
EPS = 1e-5
TEMPERATURE = 0.07

_BUILD_CACHE = {}


class _Pool:
    """Manually managed tile-pool lifetime (enter now, exit at any point)."""

    def __init__(self, tc, **kw):
        self._cm = tc.tile_pool(**kw)
        self.pool = self._cm.__enter__()
        self._open = True

    def tile(self, *a, **kw):
        if "name" not in kw:
            kw["name"] = kw.get("tag") or f"t{id(self) % 9973}"
        return self.pool.tile(*a, **kw)

    def close(self):
        if self._open:
            self._cm.__exit__(None, None, None)
            self._open = False


def _build(c_scale: float, trivial: bool, s: int = S, d: int = D):
    """Builds the single-core Bass program. Returns the compiled Bacc module."""
    import concourse.bass as bass
    import concourse.tile as tile
    from concourse import bacc, mybir

    f32 = mybir.dt.float32
    bf16 = mybir.dt.bfloat16
    f8 = mybir.dt.float8e4
    DR = mybir.MatmulPerfMode.DoubleRow
    AF = mybir.ActivationFunctionType
    AX = mybir.AxisListType
    OP = mybir.AluOpType

    st_n = s // P          # number of 128-row tiles over S
    dt_n = d // P          # number of 128-row tiles over D
    kt_n = 2 * dt_n        # k tiles over 2D for the output projection
    ch = 512               # matmul free-dim chunk (one PSUM bank of fp32)
    tc_n = s // ch         # chunks over S
    g_sz = ch // P         # s-tiles per 512-column group
    dw = d + DPAD          # weight width incl. mean column + pad
    d_chunks = [(i, min(ch, dw - i)) for i in range(0, dw, ch)]

    nc = bacc.Bacc(
        "TRN2",
        target_bir_lowering=False,
        debug=False,
        enable_asserts=False,
        num_devices=B,
    )

    vlT_d = nc.dram_tensor("vlT", [2 * d, s], bf16, kind="ExternalInput")
    vW_d = nc.dram_tensor("vW", [d, dw], bf16, kind="ExternalInput")
    lW_d = nc.dram_tensor("lW", [d, dw], bf16, kind="ExternalInput")
    oW_d = nc.dram_tensor("oW", [2 * d, dw], bf16, kind="ExternalInput")
    if not trivial:
        vb_d = nc.dram_tensor("vb", [1, dw], bf16, kind="ExternalInput")
        lb_d = nc.dram_tensor("lb", [1, dw], bf16, kind="ExternalInput")
        ob_d = nc.dram_tensor("ob", [1, dw], bf16, kind="ExternalInput")
        vg_d = nc.dram_tensor("vg", [1, d], f32, kind="ExternalInput")
        vbe_d = nc.dram_tensor("vbeta", [1, d], f32, kind="ExternalInput")
        lg_d = nc.dram_tensor("lg", [1, d], f32, kind="ExternalInput")
        lbe_d = nc.dram_tensor("lbeta", [1, d], f32, kind="ExternalInput")
        og_d = nc.dram_tensor("og", [1, d], f32, kind="ExternalInput")
        obe_d = nc.dram_tensor("obeta", [1, d], f32, kind="ExternalInput")
    out_d = nc.dram_tensor("out", [s, d], f32, kind="ExternalOutput")

    with tile.TileContext(nc) as tc:
        pp = _Pool(tc, name="persist", bufs=1)
        sp = _Pool(tc, name="small", bufs=4)

        eps_sb = pp.tile([P, 1], f32, tag='eps')
        nc.vector.memset(eps_sb[:], EPS)
        racc = pp.tile([P, st_n, tc_n], f32, tag='racc')
        rinv_all = pp.tile([P, st_n], f32, tag='rinv_all')
        sqt_p = _Pool(tc, name="sqt", bufs=3)

        if not trivial:
            ones_sb = pp.tile([1, P], bf16, tag="ones_sb")
            nc.vector.memset(ones_sb[:], 1.0)
            b_sb = {}
            aff = {}
            for nm, dd in (("vb", vb_d), ("lb", lb_d), ("ob", ob_d)):
                t = pp.tile([1, dw], bf16, tag=nm)
                nc.sync.dma_start(out=t[:], in_=dd.ap())
                b_sb[nm] = t
            for nm, dd in (("vg", vg_d), ("vbeta", vbe_d), ("lg", lg_d),
                           ("lbeta", lbe_d), ("og", og_d), ("obeta", obe_d)):
                t = pp.tile([P, d], f32, tag=nm)
                src = bass.AP(tensor=dd.ap().tensor, offset=0,
                              ap=[[0, P], [1, d]])
                nc.sync.dma_start(out=t[:], in_=src)
                aff[nm] = t

        # Address reuse via same-tag slot cycling (bufs=1):
        #   slab: vlT_sb -> E_all
        # pT_all doubles as combined^T: av^T/al^T overwrite the dead columns
        # of vpT/lpT in place (fine-grained WAR tracked by Tile).
        slab = _Pool(tc, name="slab", bufs=1)
        slab48 = _Pool(tc, name="slab48", bufs=1)
        vplp_p = _Pool(tc, name="vplp", bufs=1)
        w_p = _Pool(tc, name="wproj", bufs=2)
        ps_p = _Pool(tc, name="psuni", bufs=4, space=bass.MemorySpace.PSUM)

        def psum_tile(n):
            # one shared slot size (2 banks) for every phase: slot-level
            # recycling instead of pool-boundary barriers
            t = ps_p.tile([P, dw], f32, tag="ps")
            return t[:, :n]

        vp_all = vplp_p.tile([P, st_n, d], bf16, tag='vp_all')
        lp_all = vplp_p.tile([P, st_n, d], bf16, tag='lp_all')
        lp8 = vplp_p.tile([P, st_n, d], f8, tag='lp8')
        pT_all = slab48.tile([P, 2 * dt_n, s], bf16, tag='slab48',
                             name='pT_all')
        vpT_all = pT_all[:, :dt_n, :]
        lpT_all = pT_all[:, dt_n:, :]

        inv_sqrt_d = 1.0 / float(np.sqrt(d))

        def layernorm_relu_pair(pss, dsts, g_nm, be_nm, tagsfx,
                                dst_f32=False):
            """LN+relu over [:, :d] of a pair of psum tiles (mean precomputed
            in column d by the matmul). Small [P,·] scalars are batched
            across the pair to halve fixed op overheads.

            Engine split: ScalarE does the big Square pass (fused E[y^2]
            accumulate) + sqrt; VectorE does the small scalars and the
            scale/bias/relu passes.
            """
            n = len(pss)
            ssq = sp.tile([P, 2], f32, tag="ssq" + tagsfx)
            mcp = sp.tile([P, 2], f32, tag="mcp" + tagsfx)
            sqts = []
            for i, ps in enumerate(pss):
                sqt = sqt_p.tile([P, d], bf16, tag="sqt")
                nc.scalar.activation(out=sqt[:], in_=ps[:, :d],
                                     func=AF.Square, scale=inv_sqrt_d,
                                     accum_out=ssq[:, i:i + 1])
                nc.scalar.activation(out=mcp[:, i:i + 1], in_=ps[:, d:d + 1],
                                     func=AF.Copy)
                sqts.append(sqt)
            var = sp.tile([P, 2], f32, tag="var" + tagsfx)
            nc.vector.tensor_tensor(out=var[:, :n], in0=mcp[:, :n],
                                    in1=mcp[:, :n], op=OP.mult)
            nc.vector.tensor_tensor(out=var[:, :n], in0=ssq[:, :n],
                                    in1=var[:, :n], op=OP.subtract)
            rstd = sp.tile([P, 2], f32, tag="rstd" + tagsfx)
            nc.scalar.activation(out=rstd[:, :n], in_=var[:, :n],
                                 func=AF.Sqrt, bias=eps_sb[:])
            nc.vector.reciprocal(out=rstd[:, :n], in_=rstd[:, :n])
            mr = sp.tile([P, 2], f32, tag="mr" + tagsfx)
            nc.vector.tensor_tensor(out=mr[:, :n], in0=mcp[:, :n],
                                    in1=rstd[:, :n], op=OP.mult)
            nmr = sp.tile([P, 2], f32, tag="nmr" + tagsfx)
            nc.vector.tensor_scalar(out=nmr[:, :n], in0=mr[:, :n],
                                    scalar1=-1.0, scalar2=None, op0=OP.mult)
            for i, (ps, dst) in enumerate(zip(pss, dsts)):
                if trivial:
                    if i % 2 == 1:
                        # odd tile of the pair: fused relu on ScalarE to
                        # halve the VectorE tail latency
                        nc.scalar.activation(out=dst, in_=ps[:, :d],
                                             func=AF.Relu,
                                             bias=nmr[:, i:i + 1],
                                             scale=rstd[:, i:i + 1])
                        continue
                    if dst_f32:
                        tmp = ot_p.tile([P, d], f32, tag="tmpf", bufs=2)
                    else:
                        tmp = sqts[i]
                    nc.vector.tensor_scalar(out=tmp[:], in0=ps[:, :d],
                                            scalar1=rstd[:, i:i + 1],
                                            scalar2=mr[:, i:i + 1],
                                            op0=OP.mult, op1=OP.subtract)
                    nc.vector.tensor_scalar_max(out=dst, in0=tmp[:],
                                                scalar1=0.0)
                else:
                    nrm = sp.tile([P, d], f32, tag="nrm" + tagsfx, bufs=2)
                    nc.vector.tensor_scalar(out=nrm[:], in0=ps[:, :d],
                                            scalar1=rstd[:, i:i + 1],
                                            scalar2=mr[:, i:i + 1],
                                            op0=OP.mult, op1=OP.subtract)
                    nc.vector.tensor_mul(out=nrm[:], in0=nrm[:],
                                         in1=aff[g_nm][:])
                    nc.vector.tensor_add(out=nrm[:], in0=nrm[:],
                                         in1=aff[be_nm][:])
                    nc.vector.tensor_scalar_max(out=dst, in0=nrm[:],
                                                scalar1=0.0)

        def linear_into_psum(ps, x_sb, W_sb, bias_nm, n_k):
            for c0, cl in d_chunks:
                for kt in range(n_k):
                    nc.tensor.matmul(
                        ps[:, c0:c0 + cl],
                        x_sb(kt),
                        W_sb[:, kt, c0:c0 + cl],
                        start=(kt == 0),
                        stop=(kt == n_k - 1 and trivial),
                    )
                if not trivial:
                    nc.tensor.matmul(
                        ps[:, c0:c0 + cl], ones_sb[:1, :],
                        b_sb[bias_nm][:1, c0:c0 + cl],
                        start=False, stop=True)

        # ---------- phase A/B: projections ----------
        # packed v/l input; first halves of the v k-tiles land first so the
        # first matmul can start ASAP
        vlT_sb = slab.tile([P, 2 * dt_n, s], bf16, tag="slab",
                           name="vlT_sb")
        # all input loads up front on the sync ring, l k-tiles first
        # (projection order is l then v), first halves before second halves
        for h in range(2):
            for j in list(range(dt_n, 2 * dt_n)) + list(range(dt_n)):
                nc.sync.dma_start(
                    out=vlT_sb[:, j, h * (s // 2):(h + 1) * (s // 2)],
                    in_=vlT_d.ap()[j * P:(j + 1) * P,
                                   h * (s // 2):(h + 1) * (s // 2)])

        def proj(base, W_d, xp_all, xpT_all, bias_nm, g_nm, be_nm):
            W_sb = w_p.tile([P, dt_n, dw], bf16, tag="wproj", name="W_sb")
            for j in range(dt_n):
                nc.gpsimd.dma_start(out=W_sb[:, j, :],
                                    in_=W_d.ap()[j * P:(j + 1) * P, :])
            for st0 in range(0, st_n, 2):
                pss, dsts = [], []
                for st in (st0, st0 + 1):
                    ps = psum_tile(dw)
                    linear_into_psum(
                        ps,
                        lambda kt: vlT_sb[:, base + kt, st * P:(st + 1) * P],
                        W_sb, bias_nm, dt_n)
                    pss.append(ps)
                    dsts.append(xp_all[:, st, :])
                layernorm_relu_pair(pss, dsts, g_nm, be_nm, "p")
                for st in (st0, st0 + 1):
                    nc.sync.dma_start_transpose(
                        out=xpT_all[:, :, st * P:(st + 1) * P],
                        in_=xp_all[:, st, :])

        proj(dt_n, lW_d, lp_all, lpT_all, "lb", "lg", "lbeta")
        proj(0, vW_d, vp_all, vpT_all, "vb", "vg", "vbeta")
        w_p.close()

        # ---------- phase C+D interleaved (D lags one pair) ----------
        # C: sim row-tile st -> E (exp with accumulated row sums); E rows are
        # then rescaled by the softmax normalizer into En (so phase D needs
        # no rinv broadcast at all).
        # D (per pair of row tiles q): E^T transposes of En -> av^T columns
        # written over vpT's dead columns.
        E8 = slab.tile([P, st_n, s], f8, tag="slab", name="E8")
        g2 = 2 * P                    # s-columns per D group (2 row tiles)
        at_p = _Pool(tc, name="at", bufs=2)
        en_p = _Pool(tc, name="en", bufs=2)
        en_tiles = {}
        rinvK = pp.tile([P, st_n], f32, tag='rinvK')

        def phase_c(st):
            for t0 in range(tc_n):
                ps = psum_tile(ch)
                for dt in range(dt_n):
                    nc.tensor.matmul(
                        ps[:],
                        vpT_all[:, dt, st * P:(st + 1) * P],
                        lpT_all[:, dt, t0 * ch:(t0 + 1) * ch],
                        start=(dt == 0), stop=(dt == dt_n - 1))
                # fp8 E, stored per-256-block t-pair-interleaved so the
                # DoubleRow ifmap of phase E reads it as 3D blocks
                eout = E8[:, st, t0 * ch:(t0 + 1) * ch].rearrange(
                    "p (c q two) -> p c two q", c=2, q=P, two=2)
                nc.scalar.activation(
                    out=eout, in_=ps[:], func=AF.Exp, scale=float(c_scale),
                    accum_out=racc[:, st, t0:t0 + 1])
            rs = sp.tile([P, 1], f32, tag="rs")
            nc.vector.tensor_reduce(out=rs[:], in_=racc[:, st, :],
                                    axis=AX.X, op=OP.add)
            nc.vector.reciprocal(out=rinv_all[:, st:st + 1], in_=rs[:])
            nc.vector.tensor_scalar(out=rinvK[:, st:st + 1],
                                    in0=rinv_all[:, st:st + 1],
                                    scalar1=1024.0, scalar2=None, op0=OP.mult)
            # fold scaled-up 1/Z into fp8 lp rows (plain 1/Z would underflow
            # fp8; the al^T evacuation divides the 1024 back out)
            nc.vector.tensor_scalar_mul(
                out=lp8[:, st, :], in0=lp_all[:, st, :],
                scalar1=rinvK[:, st:st + 1])
            # row-normalized bf16 E copy for the A @ vp path
            q = st // 2
            if st % 2 == 0:
                en_tiles[q] = en_p.tile([P, 2, s], bf16, tag="en")
            esrc = E8[:, st, :].rearrange("p (c q two) -> p c two q",
                                          c=s // (2 * P), q=P, two=2)
            edst = en_tiles[q][:, st % 2, :].rearrange(
                "p (c two q) -> p c two q", c=s // (2 * P), two=2, q=P)
            nc.vector.tensor_scalar_mul(
                out=edst, in0=esrc, scalar1=rinv_all[:, st:st + 1])

        def phase_d(q):
            en = en_tiles.pop(q)
            at = at_p.tile([P, st_n, g2], bf16, tag="atg")
            for i in range(2):
                nc.sync.dma_start_transpose(
                    out=at[:, :, i * P:(i + 1) * P],
                    in_=en[:, i, :])
            for dt in range(dt_n):
                ps = psum_tile(g2)
                for tt in range(st_n):
                    nc.tensor.matmul(
                        ps[:],
                        vp_all[:, tt, dt * P:(dt + 1) * P],
                        at[:, tt, :],
                        start=(tt == 0), stop=(tt == st_n - 1))
                # psum -> av^T in vpT's dead columns (already 1/Z-scaled)
                nc.vector.tensor_copy(
                    out=pT_all[:, dt, q * g2:(q + 1) * g2], in_=ps[:])

        for st in range(st_n):
            phase_c(st)
            if st % 2 == 1 and st >= 3:
                phase_d((st - 3) // 2)
        phase_d(st_n // 2 - 1)
        en_p.close()
        at_p.close()

        # ---------- phase E: al^T = (lp')^T E ; phase F: output projection --
        ow_p = _Pool(tc, name="wout", bufs=1)
        ot_p = _Pool(tc, name="outsb", bufs=2)
        oW_sb = ow_p.tile([P, kt_n, dw], bf16, tag="wout", name="oW_sb")
        for j in range(kt_n):
            nc.gpsimd.dma_start(out=oW_sb[:, j, :],
                                in_=oW_d.ap()[j * P:(j + 1) * P, :])

        for t0 in range(tc_n):
            for dt in range(dt_n):
                ps = psum_tile(ch)
                for q in range(st_n // 2):
                    # 3D rhs free pattern: stream the stored (c, 2*q2+two)
                    # order; psum columns come out t-permuted and are
                    # unpermuted by the evacuation AP below
                    erhs = E8[:, 2 * q:2 * q + 2,
                              t0 * ch:(t0 + 1) * ch].rearrange(
                        "p a (c f) -> p a c f", c=2, f=g2)
                    nc.tensor.matmul(
                        ps[:],
                        lp8[:, 2 * q:2 * q + 2, dt * P:(dt + 1) * P],
                        erhs,
                        start=(q == 0), stop=(q == st_n // 2 - 1),
                        perf_mode=DR)
                # al^T over lpT's dead columns; divide out the 1024 from
                # the scaled-up lp fold. Columns stay pair-interleaved
                # (contiguous store); the output projection's lhsT AP
                # unpermutes them.
                nc.vector.tensor_scalar_mul(
                    out=pT_all[:, dt_n + dt, t0 * ch:(t0 + 1) * ch],
                    in0=ps[:], scalar1=1.0 / 1024.0)
            def comb_lhsT(kt, rt):
                if kt < dt_n:
                    return pT_all[:, kt, rt * P:(rt + 1) * P]
                # al half: columns are stored (c, 2*q + two)-interleaved
                # within each 512-chunk
                tq, r = divmod(rt, g_sz)
                cc, two = divmod(r, 2)
                blk = pT_all[:, kt, tq * ch:(tq + 1) * ch].rearrange(
                    "p (c q two) -> p c two q", c=2, q=P, two=2)
                return blk[:, cc, two, :]

            for rt0 in range(t0 * g_sz, (t0 + 1) * g_sz, 2):
                pss, ots = [], []
                for rt in (rt0, rt0 + 1):
                    ps = psum_tile(dw)
                    linear_into_psum(
                        ps, lambda kt, rt=rt: comb_lhsT(kt, rt),
                        oW_sb, "ob", kt_n)
                    pss.append(ps)
                    ots.append(ot_p.tile([P, d], f32, tag="ot"))
                layernorm_relu_pair(pss, [o[:] for o in ots],
                                    "og", "obeta", "o", dst_f32=True)
                for i, rt in enumerate((rt0, rt0 + 1)):
                    nc.sync.dma_start(
                        out=out_d.ap()[rt * P:(rt + 1) * P, :], in_=ots[i][:])
        ot_p.close()
        ow_p.close()
        ps_p.close()
        vplp_p.close()
        slab48.close()
        slab.close()
        sqt_p.close()
        sp.close()
        pp.close()

    nc.compile()
    return nc


def _get_program(c_scale: float, trivial: bool, s: int = S, d: int = D):
    key = (round(float(c_scale), 12), trivial, s, d)
    if key not in _BUILD_CACHE:
        _BUILD_CACHE[key] = _build(c_scale, trivial, s, d)
    return _BUILD_CACHE[key]


def _with_mean_col(W):
    """[K, N] weights -> [K, N + DPAD] bf16 with col N = row-mean, pad 0."""
    W = np.asarray(W, np.float32)
    k = W.shape[0]
    ext = np.zeros((k, W.shape[1] + DPAD), np.float32)
    ext[:, :W.shape[1]] = W
    ext[:, W.shape[1]] = W.mean(axis=1)
    return np.ascontiguousarray(ext.astype(BF16))


def _prep_in_maps(vision, language, vW, lW, oW, c_scale, trivial, extras):
    n_b = vision.shape[0]
    vWb = _with_mean_col(vW)
    lWb = _with_mean_col(lW)
    oWb = _with_mean_col(oW)
    in_maps = []
    for b in range(n_b):
        vlT = np.concatenate([vision[b].T, language[b].T], 0)
        m = {
            "vlT": np.ascontiguousarray(vlT.astype(BF16)),
            "vW": vWb, "lW": lWb, "oW": oWb,
        }
        if not trivial:
            m.update(extras)
        in_maps.append(m)
    return in_maps


def _program_and_inmaps(inputs):
    """(compiled program, per-core input maps) for the given full inputs."""
    vision = np.asarray(inputs["vision_features"], np.float32)
    language = np.asarray(inputs["language_features"], np.float32)
    c_scale = float(np.asarray(inputs["claw"], np.float32).mean()) / TEMPERATURE
    nc = _get_program(c_scale, True)
    in_maps = _prep_in_maps(vision, language, inputs["vW"], inputs["lW"],
                            inputs["oW"], c_scale, True, {})
    return nc, in_maps


def kernel(vision_features, language_features, vW, vb, vg, vbeta,
           lW, lb, lg, lbeta, claw, oW, ob, og, obeta):
    from concourse import bass_utils

    vision = np.asarray(vision_features, np.float32)
    language = np.asarray(language_features, np.float32)
    c_scale = float(np.asarray(claw, np.float32).mean()) / TEMPERATURE
    # softmax is computed without max-subtraction: guard that exp can't
    # overflow (|logit| <= |c| * max|sim|; rows have L2 norm <~ sqrt(D))
    assert abs(c_scale) * 1.5 * D < 80.0, "logit scale too large for exp"

    trivial = (
        np.all(np.asarray(vb) == 0) and np.all(np.asarray(lb) == 0)
        and np.all(np.asarray(ob) == 0)
        and np.all(np.asarray(vg) == 1) and np.all(np.asarray(vbeta) == 0)
        and np.all(np.asarray(lg) == 1) and np.all(np.asarray(lbeta) == 0)
        and np.all(np.asarray(og) == 1) and np.all(np.asarray(obeta) == 0)
    )

    def bias_ext(bv):
        bv = np.asarray(bv, np.float32).reshape(D)
        ext = np.zeros(D + DPAD, np.float32)
        ext[:D] = bv
        ext[D] = bv.mean()
        return ext.reshape(1, D + DPAD).astype(BF16)

    extras = {}
    if not trivial:
        extras = {
            "vb": bias_ext(vb), "lb": bias_ext(lb), "ob": bias_ext(ob),
            "vg": np.asarray(vg, np.float32).reshape(1, D),
            "vbeta": np.asarray(vbeta, np.float32).reshape(1, D),
            "lg": np.asarray(lg, np.float32).reshape(1, D),
            "lbeta": np.asarray(lbeta, np.float32).reshape(1, D),
            "og": np.asarray(og, np.float32).reshape(1, D),
            "obeta": np.asarray(obeta, np.float32).reshape(1, D),
        }

    nc = _get_program(c_scale, trivial)
    in_maps = _prep_in_maps(vision, language, vW, lW, oW,
                            c_scale, trivial, extras)
    res = bass_utils.run_bass_kernel_spmd(nc, in_maps,
                                          core_ids=list(range(B)))
    return np.stack([res.results[b]["out"] for b in range(B)], axis=0)
